# revision 36
# baseline (speedup 1.0000x reference)
"""Jacobi 100-step solver on 8 trn2 cores via truncated DST-spectral transform.

x_{t+1} = mask * (0.25 * 4-neighbor-sum) is linear and diagonalizes in the DST
basis Q: after one explicit step, x100 = Q (s^99 . (Q x1 Q)) Q with
s = 0.5(cos a + cos b). |s|^99 is negligible outside the lowest-K and highest-K
mode corners (K=256 -> rel err ~6.5e-3 incl fp16 noise, gate 2e-2). Everything
on-device runs in fp16 (1-pass PE matmuls, fp32 PSUM accumulation); the
spectral AllReduce payload is one [512,256] fp16 block. Sharding: 256-column
panels per core. DMA triggers cost ~600ns each serialized on the sync queue,
so loads are batched into few large rearranged transfers.
"""

import sys
import types
import numpy as np

N = 2048
NC = 8
P = N // NC          # 256 panel columns per core
K = 256              # spectral corner size per corner
K2 = 2 * K           # lo|hi concatenated
PW = P + 2           # panel width with 1-col halos
PW2 = 2 * PW         # X|Y interleaved row width
RC = N // 128        # 16 row chunks


def _install_ntff_hook():
    if "antenv.axon_hooks" in sys.modules:
        return
    mod = types.ModuleType("antenv.axon_hooks")
    mod._hook = None
    mod.set_axon_ntff_profile_hook = lambda h: setattr(mod, "_hook", h)
    mod.get_axon_ntff_profile_hook = lambda: mod._hook
    sys.modules["antenv.axon_hooks"] = mod
    try:
        import antenv
        antenv.axon_hooks = mod
        from trn_agent_boot.trn_boot import _ntff_profile_via_ctypes
        h = _ntff_profile_via_ctypes("/opt/axon/libaxon_pjrt.so")
        if h is not None:
            mod.set_axon_ntff_profile_hook(h)
    except Exception:
        pass


def _host_constants():
    # hi modes in DESCENDING order (m = 2046..1791) so that
    # Qc_hi = diag((-1)^(i+1)) @ Qc_lo (DST checkerboard identity):
    # only the lo basis is uploaded; hi is derived on-device by sign flips.
    i = np.arange(N, dtype=np.float64)
    qcs, qcTs, w99s = [], [], []
    for lo in (True, False):
        m = np.arange(1, K + 1, dtype=np.float64) if lo else np.arange(N - 2, N - 2 - K, -1, dtype=np.float64)
        red = np.outer(i, m) % (2 * (N - 1))
        Qc = np.sqrt(2.0 / (N - 1)) * np.sin(np.pi * red / (N - 1))   # [2048, K]
        lam = 0.5 * np.cos(np.pi * m / (N - 1))
        W99 = (lam[:, None] + lam[None, :]) ** 99                     # [K, K]
        qcs.append(Qc.astype(np.float16))
        qcTs.append(np.ascontiguousarray(Qc.T).astype(np.float16))
        w99s.append(W99.astype(np.float16))
    sident = np.zeros((128, 128), np.float16)
    for p in range(128):
        sident[p, p] = -1.0 if p % 2 == 0 else 1.0   # (-1)^(p+1)
    consts = {
        "qcb": qcs[0],                                                # [2048, 256] lo only
        "qcTb": np.ascontiguousarray(np.concatenate(qcTs, axis=0)),   # [512, 2048]
        "w99b": np.ascontiguousarray(np.concatenate(w99s, axis=0)),   # [512, 256]
        "ident": np.eye(128, dtype=np.float16),
        "sident": sident,
    }
    smid = np.zeros((128, 128), np.float16)
    for d in range(127):
        smid[d, d + 1] = 1.0
        smid[d + 1, d] = 1.0
    sup = np.zeros((128, 128), np.float16); sup[127, 0] = 1.0
    sdn = np.zeros((128, 128), np.float16); sdn[0, 127] = 1.0
    sgncol = np.array([[-1.0 if p % 2 == 0 else 1.0] for p in range(128)], np.float16)
    consts["sss"] = np.ascontiguousarray(np.concatenate([smid, sup, sdn, sgncol], axis=1))  # [128, 385]
    return consts


_NC_CACHE = {}


def _build():
    if "nc" in _NC_CACHE:
        return _NC_CACHE["nc"]
    import concourse.bacc as bacc
    import concourse.tile as tile
    import concourse.mybir as mybir

    F16 = mybir.dt.float16
    F32 = mybir.dt.float32
    nc = bacc.Bacc("TRN2", target_bir_lowering=False, debug=False, num_devices=NC)

    sss_d = nc.dram_tensor("sss", [128, 385], F16, kind="ExternalInput")
    xy_d = nc.dram_tensor("xy", [N, PW2], F16, kind="ExternalInput")
    qcb_d = nc.dram_tensor("qcb", [N, K], F16, kind="ExternalInput")
    qrowsb_d = nc.dram_tensor("qrowsb", [P, K], F16, kind="ExternalInput")
    w99b_d = nc.dram_tensor("w99b", [K2, K], F16, kind="ExternalInput")
    qrowsTb_d = nc.dram_tensor("qrowsTb", [K2, P], F16, kind="ExternalInput")
    ident_d = nc.dram_tensor("ident", [128, 128], F16, kind="ExternalInput")
    sident_d = nc.dram_tensor("sident", [128, 128], F16, kind="ExternalInput")
    out_d = nc.dram_tensor("out", [N, P], F16, kind="ExternalOutput")

    ACTF = mybir.ActivationFunctionType
    LN025 = float(np.log(0.25))

    with tile.TileContext(nc) as tc:
        with tc.tile_pool(name="pers", bufs=1) as pers, \
             tc.tile_pool(name="rot", bufs=6) as rot, \
             tc.tile_pool(name="ps", bufs=1, space="PSUM") as ps, \
             tc.tile_pool(name="dram", bufs=2, space="DRAM") as dram:

            # ---- persistent SBUF ----
            x0b = pers.tile([128, RC * PW], F16, tag="x0b")
            x1b = pers.tile([128, RC * P], F16, tag="x1b")
            x1db = pers.tile([128, RC * P], F16, tag="x1db")
            qcb_s = pers.tile([128, RC * K], F16, tag="qcb")
            qcTb_s = pers.tile([128, 4 * N], F16, tag="qcTb")
            qrowsb_s = pers.tile([128, 2 * K], F16, tag="qrb")
            qrowsTb_s = pers.tile([128, 4 * P], F16, tag="qrtb")
            w99_s = pers.tile([128, 4 * K], F16, tag="w99")
            abuf = pers.tile([128, 2 * K2], F16, tag="abuf")
            gsb = pers.tile([128, 4 * K], F16, tag="gsb")
            utb = pers.tile([128, 4 * K], F16, tag="utb")
            zbuf = pers.tile([128, 4 * P], F16, tag="zbuf")
            ident_s = pers.tile([128, 128], F16, tag="ident")
            halfbc = pers.tile([128, 1], F16, tag="halfbc")
            nc.gpsimd.memset(halfbc[:], 0.5)
            sident_s = pers.tile([128, 128], F16, tag="sident")
            sgn_ap = pers.tile([128, 1], F32, tag="sgnap")
            sss_s = pers.tile([128, 385], F16, tag="sss")

            # const APs for activation bias values
            for cv, cn in ((-0.5, "cneg05"), (LN025, "cln025")):
                ct = pers.tile([128, 1], F32, tag=cn, name=cn)
                nc.vector.memset(ct[:], cv)
                nc.const_aps.aps[(F32, float(cv))] = ct[:]

            # ---- start-of-kernel barrier: tiny AllReduce that runs on the idle CC
            # engine during forward; its result gates mm2 (see below) ----
            barrier_in = dram.tile([128, 1], F32, tag="barin")
            barrier_out = dram.tile([128, 1], F32, tag="barout", addr_space="Shared")
            zt = rot.tile([128, 1], F32, tag="zt")
            nc.vector.memset(zt[:], 0.0)
            nc.sync.dma_start(barrier_in[:, :], zt[:])
            nc.gpsimd.collective_compute(
                "AllReduce", mybir.AluOpType.add,
                replica_groups=[list(range(NC))],
                ins=[barrier_in.opt()], outs=[barrier_out.opt()],
            )

            # ---- stencil consts ----
            nc.sync.dma_start(sss_s[:], sss_d[:, :])
            nc.sync.dma_start(ident_s[:], ident_d[:, :])
            nc.sync.dma_start(sident_s[:], sident_d[:, :])
            signbc = sss_s[:, 384:385]
            nc.vector.tensor_copy(sgn_ap[:], signbc)
            smid_s = sss_s[:, 0:128]
            sup_s = sss_s[:, 128:256]
            sdn_s = sss_s[:, 256:384]

            # ---- fused forward: phase0 -> hsum -> phase1 -> mm1, software-pipelined
            # per row chunk so no engine FIFO head-of-line-blocks on a cross-engine
            # dependency. ----
            aps = [ps.tile([128, K], F32, tag="aacc", bufs=4, name=f"aps{jj}") for jj in range(4)]
            d2s = {}
            vpss = {}

            def st_qcb(r):
                nc.sync.dma_start(qcb_s[:, K * r:K * (r + 1)], qcb_d[128 * r:128 * (r + 1), :])

            def st_dma(r):
                # ~65-130KB per transfer: single DMA queues run ~17GB/s, so large
                # batches arrive too late; tiny ones waste ~700ns/trigger on the
                # sync queue.
                xyt = rot.tile([128, PW2], F16, tag="xyt")
                if r < 2:
                    nc.sync.dma_start(xyt[0:64, :], xy_d[128 * r:128 * r + 64, :])
                    nc.sync.dma_start(xyt[64:128, :], xy_d[128 * r + 64:128 * (r + 1), :])
                else:
                    nc.sync.dma_start(xyt[:], xy_d[128 * r:128 * (r + 1), :])
                st_qcb(r)
                return xyt

            def st_sq(r, xyt):
                xt = xyt[:, 0:PW]
                yt = xyt[:, PW:PW2]
                sqx = rot.tile([128, PW], F32, tag="sqx")
                nc.scalar.activation(sqx[:], xt, ACTF.Square, bias=-0.5, scale=1.0)
                ty = rot.tile([128, PW], F16, tag="ty")
                nc.gpsimd.tensor_sub(ty[:], yt, halfbc[:].to_broadcast((128, PW)))
                return sqx, ty

            def st_d2(r, sqx, ty):
                sqy = rot.tile([128, PW], F32, tag="sqy")
                nc.vector.tensor_mul(sqy[:], ty[:], ty[:])
                d2 = rot.tile([128, PW], F32, tag="d2")
                if r % 2 == 0:
                    nc.gpsimd.tensor_add(d2[:], sqx[:], sqy[:])
                else:
                    nc.vector.tensor_add(d2[:], sqx[:], sqy[:])
                return d2

            def st_exp(r, d2):
                # 0.25*exp(-50 d) == exp(-50 d + ln(1/4))
                nc.scalar.activation(x0b[:, PW * r:PW * (r + 1)], d2[:], ACTF.Exp,
                                     bias=LN025, scale=-50.0)

            def st_stencil(r):
                # full 4-neighbor stencil in one PSUM accumulation: vertical via the
                # shift stationaries, horizontal via identity on column-offset slices
                vps = ps.tile([128, P], F32, tag="pp", bufs=4, name="vps")
                nc.tensor.matmul(vps[:], smid_s, x0b[:, PW * r + 1:PW * r + 1 + P],
                                 start=True, stop=False)
                nc.tensor.matmul(vps[:], ident_s[:], x0b[:, PW * r:PW * r + P],
                                 start=False, stop=False)
                if r > 0:
                    nc.tensor.matmul(vps[:], sup_s, x0b[:, PW * (r - 1) + 1:PW * (r - 1) + 1 + P],
                                     start=False, stop=False)
                if r < RC - 1:
                    nc.tensor.matmul(vps[:], sdn_s, x0b[:, PW * (r + 1) + 1:PW * (r + 1) + 1 + P],
                                     start=False, stop=False)
                nc.tensor.matmul(vps[:], ident_s[:], x0b[:, PW * r + 2:PW * r + 2 + P],
                                 start=False, stop=True)
                return vps

            def st_x1(r, vps):
                nc.vector.tensor_copy(x1b[:, P * r:P * (r + 1)], vps[:])

            def st_x1d(r):
                sl = slice(P * r, P * (r + 1))
                if r % 2 == 0:
                    nc.gpsimd.tensor_mul(x1db[:, sl], x1b[:, sl], signbc.to_broadcast((128, P)))
                else:
                    nc.scalar.activation(x1db[:, sl], x1b[:, sl], ACTF.Copy, bias=0.0,
                                         scale=sgn_ap[:])

            def st_mm1(r):
                for jm in range(2):
                    nc.tensor.matmul(aps[2 * jm][:],
                                     x1b[:, P * r + 128 * jm:P * r + 128 * (jm + 1)],
                                     qcb_s[:, K * r:K * (r + 1)],
                                     start=(r == 0), stop=(r == RC - 1))
                    nc.tensor.matmul(aps[2 * jm + 1][:],
                                     x1db[:, P * r + 128 * jm:P * r + 128 * (jm + 1)],
                                     qcb_s[:, K * r:K * (r + 1)],
                                     start=(r == 0), stop=(r == RC - 1))

            xyts = {}
            for r in range(RC + 5):
                if 0 <= r - 2 < RC:
                    st_exp(r - 2, d2s.pop(r - 2))
                if r < RC:
                    xyts[r] = st_dma(r)
                    d2s[r] = st_sq(r, xyts.pop(r))
                if 0 <= r - 1 < RC:
                    d2s[r - 1] = st_d2(r - 1, *d2s[r - 1])
                if 0 <= r - 3 < RC:
                    vpss[r - 3] = st_stencil(r - 3)
                if 0 <= r - 4 < RC:
                    st_x1(r - 4, vpss.pop(r - 4))
                    st_x1d(r - 4)
                if 0 <= r - 5 < RC:
                    st_mm1(r - 5)

            # ---- remaining const loads: one trigger each (streamed during
            # late-forward / AllReduce window) ----
            nc.sync.dma_start(qrowsb_s[:].rearrange("p (c w) -> p c w", w=K2),
                              qrowsb_d[:, :].rearrange("(c p) w -> p c w", p=128))
            nc.sync.dma_start(w99_s[:].rearrange("p (c w) -> p c w", w=K),
                              w99b_d[:, :].rearrange("(c p) w -> p c w", p=128))
            nc.sync.dma_start(qrowsTb_s[:].rearrange("p (c w) -> p c w", w=P),
                              qrowsTb_d[:, :].rearrange("(c p) w -> p c w", p=128))
            nc.sync.dma_start(ident_s[:], ident_d[:, :])
            nc.sync.dma_start(sident_s[:], sident_d[:, :])
            signbc = sss_s[:, 384:385]
            nc.vector.tensor_copy(sgn_ap[:], signbc)

            for jm in range(2):
                nc.vector.tensor_copy(abuf[:, K2 * jm:K2 * jm + K], aps[2 * jm][:])
                nc.vector.tensor_mul(abuf[:, K2 * jm + K:K2 * (jm + 1)], aps[2 * jm + 1][:],
                                     signbc.to_broadcast((128, K)))

            # ---- mm2: G_t = A_t^T @ Qrows_t -> DRAM for AllReduce (fp16 payload) ----
            gin = dram.tile([K2, K], F16, tag="gin")
            gout = dram.tile([K2, K], F16, tag="gout", addr_space="Shared")
            for ti in range(2):
                for am in range(2):
                    gps = ps.tile([128, K], F32, tag="pp", bufs=4, name="gps")
                    for kj in range(2):
                        nc.tensor.matmul(gps[:],
                                         abuf[:, K2 * kj + K * ti + 128 * am:K2 * kj + K * ti + 128 * (am + 1)],
                                         qrowsb_s[:, K * kj:K * (kj + 1)],
                                         start=(kj == 0), stop=(kj == 1))
                    b = 2 * ti + am
                    nc.scalar.copy(gsb[:, K * b:K * (b + 1)], gps[:])
            nc.sync.dma_start(gin[:, :].rearrange("(c p) w -> p c w", p=128),
                              gsb[:].rearrange("p (c w) -> p c w", w=K))
            nc.gpsimd.collective_compute(
                "AllReduce", mybir.AluOpType.add,
                replica_groups=[list(range(NC))],
                ins=[gin.opt()], outs=[gout.opt()],
            )

            # ---- build QcT in SBUF from qcb via PE transposes: runs in the
            # AllReduce dead window, replacing a 2MB host upload ----
            for b in range(4):
                rhs_id = ident_s if b < 2 else sident_s
                for r in range(RC):
                    pst = ps.tile([128, P], F32, tag="pp", bufs=4, name="pst")
                    nc.tensor.matmul(pst[:, 0:128],
                                     qcb_s[:, K * r + 128 * (b % 2):K * r + 128 * (b % 2 + 1)],
                                     rhs_id[:], start=True, stop=True)
                    if r % 2 == 0:
                        nc.vector.tensor_copy(qcTb_s[:, N * b + 128 * r:N * b + 128 * (r + 1)], pst[:, 0:128])
                    else:
                        nc.scalar.copy(qcTb_s[:, N * b + 128 * r:N * b + 128 * (r + 1)], pst[:, 0:128])

            # ---- spectral filter: load G^T via XBAR DMA-transpose, then W99 mul.
            # W99 is symmetric, so the same w99 chunks filter the transposed layout.
            for ti in range(2):
                for bm in range(2):
                    b = 2 * ti + bm
                    traw = rot.tile([128, K], F16, tag="traw")
                    nc.sync.dma_start_transpose(traw[:], gout[K * ti:K * (ti + 1), 128 * bm:128 * (bm + 1)])
                    nc.vector.tensor_mul(utb[:, K * b:K * (b + 1)], traw[:],
                                         w99_s[:, K * b:K * (b + 1)])

            # ---- B1: Z_t = Uhat_t @ QrowsT_t ----
            for ti in range(2):
                for am in range(2):
                    zps = ps.tile([128, P], F32, tag="pp", bufs=4, name="zps")
                    for kb in range(2):
                        nc.tensor.matmul(zps[:],
                                         utb[:, K * (2 * ti + kb) + 128 * am:K * (2 * ti + kb) + 128 * (am + 1)],
                                         qrowsTb_s[:, P * (2 * ti + kb):P * (2 * ti + kb + 1)],
                                         start=(kb == 0), stop=(kb == 1))
                    nc.vector.tensor_copy(zbuf[:, P * (2 * ti + am):P * (2 * ti + am + 1)], zps[:])

            # ---- B2: out_r = sum_{t,ka} QcT_{t,ka,r}^T @ Z_{t,ka}; DMA out in row pairs ----
            for re in range(RC // 2):
                osb2 = rot.tile([128, 2 * P], F16, tag="osb", name="osb2")
                for half in range(2):
                    r = 2 * re + half
                    ops = ps.tile([128, P], F32, tag="pp", bufs=4, name="ops")
                    for ti in range(2):
                        for ka in range(2):
                            b = 2 * ti + ka
                            nc.tensor.matmul(ops[:],
                                             qcTb_s[:, N * b + 128 * r:N * b + 128 * (r + 1)],
                                             zbuf[:, P * b:P * (b + 1)],
                                             start=(b == 0), stop=(b == 3))
                    if half == 0:
                        nc.vector.tensor_copy(osb2[:, 0:P], ops[:])
                    else:
                        nc.scalar.copy(osb2[:, P:2 * P], ops[:])
                nc.scalar.dma_start(out_d[256 * re:256 * re + 128, :], osb2[:, 0:P])
                nc.scalar.dma_start(out_d[256 * re + 128:256 * (re + 1), :], osb2[:, P:2 * P])

    nc.compile()
    _NC_CACHE["nc"] = nc
    return nc


def _run(X, Y, trace=False):
    _install_ntff_hook()
    from concourse.bass_utils import run_bass_kernel_spmd

    consts = _host_constants()
    Xp = np.zeros((N, N + 2), np.float16); Xp[:, 1:-1] = np.asarray(X, np.float32).astype(np.float16)
    Yp = np.zeros((N, N + 2), np.float16); Yp[:, 1:-1] = np.asarray(Y, np.float32).astype(np.float16)

    in_maps = []
    for c in range(NC):
        xy = np.concatenate([Xp[:, P * c:P * c + PW], Yp[:, P * c:P * c + PW]], axis=1)
        m = {"xy": np.ascontiguousarray(xy),
             "qcb": consts["qcb"],
             "w99b": consts["w99b"],
             "qrowsb": np.ascontiguousarray(consts["qcb"][P * c:P * (c + 1), :]),
             "sident": consts["sident"],
             "qrowsTb": np.ascontiguousarray(consts["qcTb"][:, P * c:P * (c + 1)]),
             "sss": consts["sss"],
             "ident": consts["ident"]}
        in_maps.append(m)

    nc = _build()
    r = run_bass_kernel_spmd(nc, in_maps, core_ids=list(range(NC)), trace=trace)
    panels = [r.results[c]["out"] for c in range(NC)]
    full = np.concatenate(panels, axis=1).astype(np.float32)
    return full[None, None], r


def kernel(X, Y):
    out, _ = _run(X, Y, trace=False)
    return out
